# revision 21
# baseline (speedup 1.0000x reference)
"""Fused DoubleXLSTMDown kernel for 8 Trainium2 NeuronCores.

Sharding: data-parallel over batch (B=8 -> 1 batch item per core). Weights are
broadcast; the host pre-casts them to fp8e4 (projections) / bf16 and
pre-rearranges into DMA-friendly, matmul-ready layouts. No collectives.

v2: all projection matmuls (LN stats, up, conv, gates, q/k/v, q@kT, down) run
in fp8e4 with DoubleRow perf mode, halving PE streaming time per contraction
chain. The attention scores stay bf16 (the u=exp(a-G) decomposition spans an
exponent range far beyond fp8), so scores@v stays a bf16 chain. Static
power-of-2 scales keep every fp8 tensor in range; scales fold into constants
(u-bias, silu input scales, folded gains) so no extra passes are needed.

On-chip layout: activations are kept feature-major [feat, seq] so every
projection is `out = W.T @ act` with W in its natural layout as lhsT; the
attention value/output path runs seq-major (group-norm needs free-dim stats).
The mLSTM decay matrix uses the exact decomposition
    D[t,s] = exp(a[s] - M[t]) = exp(a[s]-G) * exp(G-M[t]),
    a = ipre + spc,  spc = cumsum(softplus(-fpre)),  M = runmax(a), G = max(a)
so no [S,S] row-max/exp is needed: u=exp(a-G) folds into the score-tile copy,
r=exp(G-M) folds into the per-row normalizer; 1-D scans run on the DVE.

The depthwise conv runs as fp8 DR diagonal matmuls: x_m is evicted twice into
a [P, IC, 2, 1040] tile (copy1 shifted left by one column), so a tap pair
(j, j+1) is one DR matmul whose rhs k-tiles are the two shifted windows.
"""

import math
import threading

import numpy as np
import ml_dtypes

import concourse.bass as bass  # noqa: F401
import concourse.mybir as mybir
import concourse.tile as tile
from concourse import bacc
from concourse.masks import make_identity
from concourse.bass_utils import run_bass_kernel_spmd

P = 128
B, S, E = 8, 1024, 512
L, H, KC, F = 2, 4, 4, 128
I = 2 * E
DH = I // H      # 256
EC = E // P      # 4
IC = I // P      # 8
ST = S // P      # 8
HV = 2           # halves of 512 along seq
EPS = 1e-5
NCORES = 8
XW = 1040        # xm2 row width (16-byte aligned; data at col 16 / 15)

f32 = mybir.dt.float32
f32r = mybir.dt.float32r
bf16 = mybir.dt.bfloat16
f8 = mybir.dt.float8e4
AF = mybir.ActivationFunctionType
OP = mybir.AluOpType
DR = mybir.MatmulPerfMode.DoubleRow
BF = ml_dtypes.bfloat16
F8 = ml_dtypes.float8_e4m3

# static power-of-2 scales
S_WUP = 64.0     # wup stored *64          -> up psum = 64*up
S_XM = 8.0       # xm stored *8            (evict scale 8/64 = 1/8)
S_WCV = 16.0     # conv w stored *16       -> conv psum = 128*c_pre
S_WQ = 1024.0    # wq(incl DH^-.5) *1024   -> q_f = 1024*q  (plain evict)
S_WK = 64.0      # wk *64                  -> k_f = 64*k    (plain evict)
S_QK = S_WQ * S_WK  # qk psum scale        (folds into u bias)
S_WV = 64.0      # wv *64                  -> v psum = 512*v (evict 1/512)
S_WIF = 256.0    # gate w *256             -> gates psum = 256*pre
S_HF = 32.0      # hfin stored *32         (via gng*32 / skip*32 folds)
S_WD = 64.0      # wdown *64               -> down psum = 2048*down


def _f8(a):
    return np.clip(np.asarray(a, np.float32), -240.0, 240.0).astype(F8)


# ---------------------------------------------------------------------------
# host-side weight preparation
# ---------------------------------------------------------------------------

def _prep_weights(inp):
    w = {}
    scale = DH ** -0.5

    wq = np.asarray(inp["wq"], np.float32)
    wk = np.asarray(inp["wk"], np.float32)
    wv = np.asarray(inp["wv"], np.float32)
    w_i = np.asarray(inp["w_i"], np.float32)
    w_f = np.asarray(inp["w_f"], np.float32)

    # streamed m-tile layouts: [L, M, P, C, P] with [l, m, p, c, f] = W[l, c*P+p, m*P+f]
    def mtile(a, mt, ct):
        lw = a.reshape(L, ct, P, mt, P).transpose(0, 3, 2, 1, 4)
        return np.ascontiguousarray(lw)

    w["wup"] = _f8(mtile(np.asarray(inp["w_up"], np.float32) * S_WUP, 16, EC))
    w["wq"] = _f8(mtile(wq * scale * S_WQ, IC, IC))
    w["wk"] = _f8(mtile(wk * S_WK, IC, IC))
    w["wdown"] = _f8(mtile(np.asarray(inp["w_down"], np.float32) * S_WD, EC, IC))

    # wv: [L, half, P, C, 512] with [l,h,p,c,n] = wv[l, c*P+p, h*512+n]
    wvl = (wv * S_WV).reshape(L, IC, P, HV, 512).transpose(0, 3, 2, 1, 4)
    w["wv"] = _f8(np.ascontiguousarray(wvl))

    # depthwise-conv diag blocks: [L, P, C, KC, P], [l,p,c,j,f] = conv_w[l,j,c*P+p] iff f==p
    cw = (np.asarray(inp["conv_w"], np.float32) * S_WCV).reshape(L, KC, IC, P)
    cd = np.zeros((L, P, IC, KC, P), np.float32)
    rng = np.arange(P)
    cd[:, rng, :, :, rng] = cw.transpose(3, 0, 2, 1)  # -> [p, l, c, j]
    w["wconv"] = _f8(cd)

    # fused gate weights: [ipre|fpre](8) = [c_act | x_m/S_XM] @ wif + b
    wif = np.zeros((L, 2 * I, 8), np.float32)
    for l in range(L):
        wif[l, :I, 0:4] = wq[l] @ w_i[l, :I] + wk[l] @ w_i[l, I:2 * I]
        wif[l, :I, 4:8] = wq[l] @ w_f[l, :I] + wk[l] @ w_f[l, I:2 * I]
        wif[l, I:, 0:4] = wv[l] @ w_i[l, 2 * I:] / S_XM
        wif[l, I:, 4:8] = wv[l] @ w_f[l, 2 * I:] / S_XM
    wif *= S_WIF
    # [L, P, 16, 16]: free width 16 for the DR dim-1 stride alignment
    wifp = np.zeros((L, P, 16, 16), np.float32)
    wifp[:, :, :, 0:8] = wif.reshape(L, 16, P, 8).transpose(0, 2, 1, 3)
    w["wif"] = _f8(wifp)

    w["bif"] = np.ascontiguousarray(
        np.concatenate([np.asarray(inp["b_i"], np.float32),
                        np.asarray(inp["b_f"], np.float32)], axis=1).T)  # [8, L]

    # down bias rows for the in-chain bias matmul: [1, L, EC, P] = S_QKd * bdown
    w["bdrow"] = np.ascontiguousarray(
        (np.asarray(inp["b_down"], np.float32) * (S_HF * S_WD))
        .reshape(1, L, EC, P)).astype(BF)

    # final projection
    w["wfin"] = np.ascontiguousarray(
        np.asarray(inp["w_fin"], np.float32).reshape(EC, P, F).transpose(1, 0, 2)
    ).astype(BF)  # [P, EC, F]
    w["bfin"] = np.asarray(inp["b_fin"], np.float32).reshape(1, F).copy()

    def cols(a, c):  # [L, c*P] -> [P, L, c]
        return np.ascontiguousarray(
            np.asarray(a, np.float32).reshape(L, c, P).transpose(2, 0, 1))

    w["lng"] = cols(inp["ln_g"], EC)
    w["lnb"] = cols(inp["ln_b"], EC)
    w["convb"] = cols(inp["conv_b"], IC)
    w["skip"] = cols(np.asarray(inp["skip"], np.float32) * S_HF, IC)
    w["gng"] = cols(np.asarray(inp["gn_g"], np.float32) * S_HF, IC)

    # causal staircase mask [P, 896]: mask[s, j] = 1 iff j >= s + 384
    jj = np.arange(896)[None, :]
    ss = np.arange(P)[:, None]
    w["cmask"] = (jj >= ss + 384).astype(BF)
    return w


# ---------------------------------------------------------------------------
# device kernel
# ---------------------------------------------------------------------------

def build_nc(cfg=None, repeat=1):
    base_cfg = dict(scores_bufs=1, big_bufs=5, sv_bufs=2, wv_bufs=2,
                    w_bufs=2, ht_bufs=6)
    base_cfg.update(cfg or {})
    cfg = base_cfg
    nc = bacc.Bacc("TRN2", target_bir_lowering=False, debug=False,
                   num_devices=NCORES)

    x_d = nc.declare_dram_parameter("x", [S, E], f32, isOutput=False)
    wup_d = nc.declare_dram_parameter("wup", [L, 16, P, EC, P], f8, isOutput=False)
    wq_d = nc.declare_dram_parameter("wq", [L, IC, P, IC, P], f8, isOutput=False)
    wk_d = nc.declare_dram_parameter("wk", [L, IC, P, IC, P], f8, isOutput=False)
    wv_d = nc.declare_dram_parameter("wv", [L, HV, P, IC, 512], f8, isOutput=False)
    wdown_d = nc.declare_dram_parameter("wdown", [L, EC, P, IC, P], f8, isOutput=False)
    wconv_d = nc.declare_dram_parameter("wconv", [L, P, IC, KC, P], f8, isOutput=False)
    wif_d = nc.declare_dram_parameter("wif", [L, P, 16, 16], f8, isOutput=False)
    bif_d = nc.declare_dram_parameter("bif", [8, L], f32, isOutput=False)
    bdrow_d = nc.declare_dram_parameter("bdrow", [1, L, EC, P], bf16, isOutput=False)
    wfin_d = nc.declare_dram_parameter("wfin", [P, EC, F], bf16, isOutput=False)
    bfin_d = nc.declare_dram_parameter("bfin", [1, F], f32, isOutput=False)
    lng_d = nc.declare_dram_parameter("lng", [P, L, EC], f32, isOutput=False)
    lnb_d = nc.declare_dram_parameter("lnb", [P, L, EC], f32, isOutput=False)
    convb_d = nc.declare_dram_parameter("convb", [P, L, IC], f32, isOutput=False)
    skip_d = nc.declare_dram_parameter("skip", [P, L, IC], f32, isOutput=False)
    gng_d = nc.declare_dram_parameter("gng", [P, L, IC], f32, isOutput=False)
    cmask_d = nc.declare_dram_parameter("cmask", [P, 896], bf16, isOutput=False)
    y_d = nc.declare_dram_parameter("y", [S, F], f32, isOutput=True)

    with tile.TileContext(nc) as tc:
        const = tc.alloc_tile_pool(name="const", bufs=1)
        rpool = tc.alloc_tile_pool(name="rpool", bufs=1)
        act = tc.alloc_tile_pool(name="act", bufs=1)
        wstream = tc.alloc_tile_pool(name="wstream", bufs=cfg["w_bufs"])
        smalls = tc.alloc_tile_pool(name="smalls", bufs=1)
        ps_big = tc.alloc_tile_pool(name="ps_big", bufs=cfg["big_bufs"], space="PSUM")
        ps_sv = tc.alloc_tile_pool(name="ps_sv", bufs=cfg["sv_bufs"], space="PSUM")
        ps_sm = tc.alloc_tile_pool(name="ps_sm", bufs=1, space="PSUM")

        # ---- constants
        id_f32 = const.tile([P, P], f32)
        make_identity(nc, id_f32)
        id_bf = const.tile([P, P], bf16)
        make_identity(nc, id_bf)
        ones8 = const.tile([P, 2, P], f8)
        nc.vector.memset(ones8, 1.0 / E)     # 2^-9: exact fp8 subnormal
        onerow = const.tile([1, 512], bf16)
        nc.vector.memset(onerow, 1.0)
        cmask = const.tile([P, 896], bf16)
        nc.sync.dma_start(out=cmask, in_=cmask_d[:, :])
        lng = const.tile([P, L, EC], f32)
        nc.sync.dma_start(out=lng, in_=lng_d[:, :, :])
        lnb = const.tile([P, L, EC], f32)
        nc.sync.dma_start(out=lnb, in_=lnb_d[:, :, :])
        convb = const.tile([P, L, IC], f32)
        nc.sync.dma_start(out=convb, in_=convb_d[:, :, :])
        skipc = const.tile([P, L, IC], f32)
        nc.sync.dma_start(out=skipc, in_=skip_d[:, :, :])
        gng = const.tile([P, L, IC], f32)
        nc.sync.dma_start(out=gng, in_=gng_d[:, :, :])
        bif = const.tile([8, L], f32)
        nc.sync.dma_start(out=bif, in_=bif_d[:, :])
        bdrow = const.tile([1, L, EC, P], bf16)
        nc.sync.dma_start(out=bdrow, in_=bdrow_d[:, :, :, :])
        wfin = const.tile([P, EC, F], bf16)
        nc.sync.dma_start(out=wfin, in_=wfin_d[:, :, :])
        bfin = const.tile([P, F], f32)
        nc.gpsimd.dma_start(out=bfin, in_=bfin_d.ap().to_broadcast([P, F]))
        eps_col = const.tile([P, 1], f32)
        nc.vector.memset(eps_col, EPS)
        one_col = const.tile([P, 1], f32)
        nc.vector.memset(one_col, 1.0)

        # ---- residual, feature-major [P, EC, S] fp32, updated in place
        r_feat = rpool.tile([P, EC, S], f32)

        for _rep in range(repeat):
            # load x seq-major and transpose into r_feat
            xseq = act.tile([P, ST, E], f32, tag="stage", name="xseq")
            x_r = x_d.ap().rearrange("(t p) e -> p t e", p=P)
            for tt in range(ST):
                nc.sync.dma_start(out=xseq[:, tt:tt + 1, :],
                                  in_=x_r[:, tt:tt + 1, :])
            for h in range(HV):
                for c in range(EC):
                    pst = ps_big.tile([P, 512], f32, tag="big", name="ps_xT")
                    for k in range(4):
                        st = h * 4 + k
                        nc.tensor.transpose(
                            out=pst[:, k * P:(k + 1) * P],
                            in_=xseq[:, st, c * P:(c + 1) * P],
                            identity=id_f32)
                    nc.vector.tensor_copy(out=r_feat[:, c, h * 512:(h + 1) * 512],
                                          in_=pst)

            # ================= per-block =================
            for l in range(L):
                # ---------- layernorm (stats via fp8 DR ones-matmuls) ----------
                # per-half casts so each half's stats chain starts as soon as
                # the previous block's down-projection finishes that half
                sq = act.tile([P, EC, S], f8, tag="sq", name="sq")
                rb = act.tile([P, EC, S], f8, tag="rb", name="rb")
                for h in range(HV):
                    hsl = slice(h * 512, (h + 1) * 512)
                    nc.scalar.activation(out=sq[:, :, hsl], in_=r_feat[:, :, hsl],
                                         func=AF.Square)
                    nc.vector.tensor_copy(out=rb[:, :, hsl], in_=r_feat[:, :, hsl])

                rstd_b = smalls.tile([P, HV, 512], bf16, name="rstd_b")
                tvar = smalls.tile([P, 512], f32, name="tvar")
                xn = act.tile([P, EC, S], f8, tag="xn", name="xn")
                xnw = act.tile([P, EC, S], bf16, tag="xnw", name="xnw")
                for h in range(HV):
                    sl = slice(h * 512, (h + 1) * 512)
                    # ones8 holds 1/E: psums are mu and E[x^2] directly
                    ps_sum = ps_sv.tile([P, 512], f32, tag="sv", name="ps_lnsum")
                    ps_sq = ps_sv.tile([P, 512], f32, tag="sv", name="ps_lnsq")
                    for c2 in range(EC // 2):
                        nc.tensor.matmul(
                            ps_sum, ones8, rb[:, 2 * c2:2 * c2 + 2, sl],
                            perf_mode=DR,
                            start=(c2 == 0), stop=(c2 == EC // 2 - 1))
                    for c2 in range(EC // 2):
                        nc.tensor.matmul(
                            ps_sq, ones8, sq[:, 2 * c2:2 * c2 + 2, sl],
                            perf_mode=DR,
                            start=(c2 == 0), stop=(c2 == EC // 2 - 1))
                    nc.scalar.activation(out=tvar, in_=ps_sum, func=AF.Square)
                    nc.vector.tensor_sub(out=tvar, in0=ps_sq, in1=tvar)
                    nc.scalar.activation(out=tvar, in_=tvar, func=AF.Sqrt,
                                         bias=eps_col)
                    with nc.allow_low_precision(reason="bf16 LN rstd"):
                        nc.vector.reciprocal(out=rstd_b[:, h], in_=tvar)
                    for c in range(EC):
                        nc.vector.tensor_sub(out=xnw[:, c, sl],
                                             in0=r_feat[:, c, sl], in1=ps_sum)
                        nc.gpsimd.tensor_mul(out=xnw[:, c, sl], in0=xnw[:, c, sl],
                                             in1=rstd_b[:, h])
                        nc.scalar.activation(out=xn[:, c, sl], in_=xnw[:, c, sl],
                                             func=AF.Identity,
                                             scale=lng[:, l, c:c + 1],
                                             bias=lnb[:, l, c:c + 1])

                # ---------- up projection (fp8 DR) ----------
                # m 0..7 -> x_m (two shifted fp8 copies), m 8..15 -> z -> silu
                xm = act.tile([P, IC, 2, XW], f8, tag="xm2", name="xm")
                nc.vector.memset(xm[:, :, 0, 0:16], 0.0)
                nc.vector.memset(xm[:, :, 1, 0:15], 0.0)
                nc.vector.memset(xm[:, :, 1, XW - 1:XW], 0.0)
                sz = act.tile([P, IC, S], bf16, tag="sz", name="sz")
                for wave in range(2):
                    upw = []
                    for mi in range(8):
                        wt = wstream.tile([P, EC, P], f8, tag="wup",
                                          name="wup_t", bufs=8)
                        nc.sync.dma_start(out=wt, in_=wup_d[l, wave * 8 + mi])
                        upw.append(wt)
                    for h in range(HV):
                        for mi in range(8):
                            m = wave * 8 + mi
                            ps = ps_big.tile([P, 512], f32, tag="big", name="ps_up")
                            for c2 in range(EC // 2):
                                nc.tensor.matmul(
                                    ps, upw[mi][:, 2 * c2:2 * c2 + 2, :],
                                    xn[:, 2 * c2:2 * c2 + 2, h * 512:(h + 1) * 512],
                                    perf_mode=DR,
                                    start=(c2 == 0), stop=(c2 == EC // 2 - 1))
                            if m < IC:
                                o0 = 16 + h * 512
                                nc.vector.tensor_scalar_mul(
                                    out=xm[:, m, 0, o0:o0 + 512],
                                    in0=ps, scalar1=1.0 / S_WUP * S_XM)
                                nc.scalar.activation(
                                    out=xm[:, m, 1, o0 - 1:o0 + 511],
                                    in_=ps, func=AF.Copy,
                                    scale=1.0 / S_WUP * S_XM)
                            else:
                                nc.scalar.activation(
                                    out=sz[:, m - IC, h * 512:(h + 1) * 512],
                                    in_=ps, func=AF.Silu, scale=1.0 / S_WUP)

                # ---------- causal depthwise conv (fp8 DR diag matmuls) + silu ----------
                cact = act.tile([P, IC, S], f8, tag="cact", name="cact")
                for c in range(IC):
                    wcv = wstream.tile([P, KC, P], f8, tag="wconv", name="wconv_t",
                                       bufs=2)
                    nc.sync.dma_start(out=wcv, in_=wconv_d[l, :, c])
                    for h in range(HV):
                        ps = ps_big.tile([P, 512], f32, tag="big", name="ps_cv")
                        for jp in range(KC // 2):
                            a0 = 13 + 2 * jp + h * 512
                            nc.tensor.matmul(
                                ps, wcv[:, 2 * jp:2 * jp + 2, :],
                                xm[:, c, 0:2, a0:a0 + 512],
                                perf_mode=DR,
                                start=(jp == 0), stop=(jp == KC // 2 - 1))
                        nc.scalar.activation(
                            out=cact[:, c, h * 512:(h + 1) * 512], in_=ps,
                            func=AF.Silu, scale=1.0 / (S_WCV * S_XM),
                            bias=convb[:, l, c:c + 1])

                # ---------- gate pre-activations + scans ----------
                wif = wstream.tile([P, 16, 16], f8, tag="wif", name="wif_t")
                nc.sync.dma_start(out=wif, in_=wif_d[l])
                # compute-engine APs must start at partition 0/32/64/96: the
                # 1-D gate chain lives in base-0 [4,S]/[8,S] tiles; fpre is
                # extracted from rows 4:8 via an SBUF->SBUF DMA shuffle.
                g8 = smalls.tile([8, S], f32, name="g8")      # 0:4 ipre->a, 4:8 fpre
                f4 = smalls.tile([4, S], f32, name="f4")      # fpre -> sp -> u
                spc4 = smalls.tile([4, S], f32, name="spc4")  # spc -> em
                mr4 = smalls.tile([4, S], f32, name="mr4")    # runmax -> r
                for h in range(HV):
                    psg = ps_sm.tile([8, 512], f32, tag="sm", name="ps_g")
                    for cp in range(4):   # cact pairs
                        nc.tensor.matmul(psg, wif[:, 2 * cp:2 * cp + 2, 0:8],
                                         cact[:, 2 * cp:2 * cp + 2,
                                              h * 512:(h + 1) * 512],
                                         perf_mode=DR,
                                         start=(cp == 0), stop=False)
                    for cp in range(4):   # xm pairs (copy0 data window)
                        o0 = 16 + h * 512
                        nc.tensor.matmul(psg, wif[:, 8 + 2 * cp:10 + 2 * cp, 0:8],
                                         xm[:, 2 * cp:2 * cp + 2, 0, o0:o0 + 512],
                                         perf_mode=DR,
                                         start=False, stop=(cp == 3))
                    nc.vector.tensor_scalar(out=g8[:, h * 512:(h + 1) * 512],
                                            in0=psg, scalar1=1.0 / S_WIF,
                                            scalar2=bif[:, l:l + 1],
                                            op0=OP.mult, op1=OP.add)
                nc.sync.dma_start(out=f4, in_=g8[4:8, :])
                # sp = softplus(-fpre) (in place), spc = cumsum(sp)
                nc.scalar.activation(out=f4, in_=f4, func=AF.Exp, scale=-1.0)
                nc.scalar.activation(out=f4, in_=f4, func=AF.Ln, bias=one_col[0:4])
                nc.vector.tensor_tensor_scan(out=spc4, data0=f4, data1=f4,
                                             initial=0.0, op0=OP.add, op1=OP.bypass)
                # a = ipre + spc (overwrites ipre), Mr = runmax(a), G = Mr[-1]
                nc.vector.tensor_add(out=g8[0:4], in0=g8[0:4], in1=spc4)
                nc.vector.tensor_tensor_scan(out=mr4, data0=g8[0:4], data1=g8[0:4],
                                             initial=-3.0e38, op0=OP.max,
                                             op1=OP.bypass)
                gmax = smalls.tile([4, 1], f32, name="gmax")
                ngmax = smalls.tile([4, 1], f32, name="ngmax")
                nc.vector.tensor_copy(out=gmax, in_=mr4[:, S - 1:S])
                # u bias = -G - ln(S_QK): descales the fp8 qk psum
                nc.vector.tensor_scalar(out=ngmax, in0=gmax, scalar1=-1.0,
                                        scalar2=-math.log(S_QK),
                                        op0=OP.mult, op1=OP.add)
                # u = exp(a-G)/S_QK -> f4 (sp dead); em = exp(spc-Mr) -> spc4;
                # r = exp(G-Mr) -> mr4 in place
                nc.scalar.activation(out=f4, in_=g8[0:4], func=AF.Exp, bias=ngmax)
                nc.vector.tensor_sub(out=spc4, in0=spc4, in1=mr4)
                nc.scalar.activation(out=spc4, in_=spc4, func=AF.Exp)
                nc.scalar.activation(out=mr4, in_=mr4, func=AF.Exp, scale=-1.0,
                                     bias=gmax)
                # transpose u/r/em to seq-major: useq[:, c, 0:4]=u, 4:8=r, 8:12=em
                useq = smalls.tile([P, ST, 12], f32, name="useq")
                for c in range(ST):
                    pst = ps_sv.tile([P, 12], f32, tag="sv", name="ps_useq")
                    for qi, srct in enumerate((f4, mr4, spc4)):
                        nc.tensor.transpose(out=pst[:, qi * 4:qi * 4 + 4],
                                            in_=srct[:, c * P:(c + 1) * P],
                                            identity=id_f32[0:4, 0:4])
                    nc.vector.tensor_copy(out=useq[:, c, :], in_=pst)

                # ---------- v projection (fp8 DR, seq-major, + ones column) ----------
                v_sb = act.tile([P, ST, H, DH + 1], bf16, tag="v", name="v_sb")
                nc.vector.memset(v_sb[:, :, :, DH:DH + 1], 1.0)
                for h in range(HV):
                    wvt = wstream.tile([P, IC, 512], f8, tag="wv", name="wv_t",
                                       bufs=cfg["wv_bufs"])
                    nc.sync.dma_start(out=wvt, in_=wv_d[l, h])
                    for st in range(ST):
                        ps = ps_big.tile([P, 512], f32, tag="big", name="ps_v")
                        o0 = 16 + st * P
                        for c2 in range(IC // 2):
                            nc.tensor.matmul(
                                ps, xm[:, 2 * c2:2 * c2 + 2, 0, o0:o0 + P],
                                wvt[:, 2 * c2:2 * c2 + 2, :],
                                perf_mode=DR,
                                start=(c2 == 0), stop=(c2 == IC // 2 - 1))
                        if st % 2 == 0:
                            nc.vector.tensor_scalar_mul(
                                out=v_sb[:, st, 2 * h:2 * h + 2, 0:DH],
                                in0=ps.rearrange("p (a b) -> p a b", a=2),
                                scalar1=1.0 / (S_XM * S_WV))
                        else:
                            nc.scalar.activation(
                                out=v_sb[:, st, 2 * h:2 * h + 2, 0:DH],
                                in_=ps.rearrange("p (a b) -> p a b", a=2),
                                func=AF.Copy, scale=1.0 / (S_XM * S_WV))

                # ---------- q/k projections (fp8 DR, feature-major) ----------
                q_f = act.tile([P, IC, S], f8, tag="qf", name="q_f")
                k_f = act.tile([P, IC, S], f8, tag="kf", name="k_f")
                for dst, wdrm, wtag, eng in ((q_f, wq_d, "wq", "act"),
                                             (k_f, wk_d, "wk", "dve")):
                    for m in range(IC):
                        wt = wstream.tile([P, IC, P], f8, tag=wtag, name="wqk_t")
                        nc.sync.dma_start(out=wt, in_=wdrm[l, m])
                        for h in range(HV):
                            ps = ps_big.tile([P, 512], f32, tag="big", name="ps_qk")
                            for c2 in range(IC // 2):
                                nc.tensor.matmul(
                                    ps, wt[:, 2 * c2:2 * c2 + 2, :],
                                    cact[:, 2 * c2:2 * c2 + 2,
                                         h * 512:(h + 1) * 512],
                                    perf_mode=DR,
                                    start=(c2 == 0), stop=(c2 == IC // 2 - 1))
                            if eng == "act":
                                nc.scalar.activation(
                                    out=dst[:, m, h * 512:(h + 1) * 512], in_=ps,
                                    func=AF.Copy)
                            else:
                                nc.vector.tensor_copy(
                                    out=dst[:, m, h * 512:(h + 1) * 512], in_=ps)

                # csk = skip * c_act * S_HF, in place (after all cact readers)
                for c in range(IC):
                    if c % 2 == 0:
                        nc.vector.tensor_scalar_mul(out=cact[:, c, :],
                                                    in0=cact[:, c, :],
                                                    scalar1=skipc[:, l, c:c + 1])
                    else:
                        nc.scalar.activation(out=cact[:, c, :], in_=cact[:, c, :],
                                             func=AF.Copy,
                                             scale=skipc[:, l, c:c + 1])

                # ---------- attention ----------
                hgn = act.tile([P, ST, S], bf16, tag="stage", name="hgn")
                for hd in range(H):
                    for tc_ in range(HV):
                        stag = ("scoresA", "scoresB" + str(hd % 2))[tc_]
                        scores = act.tile(
                            [P, 4 * (tc_ + 1), 512], bf16,
                            tag=stag, name="scores")
                        ncc = 4 * (tc_ + 1)
                        for cc in range(ncc):
                            d = cc * P - tc_ * 512
                            d0 = max(d, 0)  # first needed t_local column
                            nw = 512 - d0
                            ps = ps_big.tile([P, 512], f32, tag="big", name="ps_qkT")
                            nc.tensor.matmul(
                                ps[:, 0:nw],
                                k_f[:, 2 * hd:2 * hd + 2, cc * P:(cc + 1) * P],
                                q_f[:, 2 * hd:2 * hd + 2,
                                    tc_ * 512 + d0:(tc_ + 1) * 512],
                                perf_mode=DR, start=True, stop=True)
                            ucol = useq[:, cc, hd:hd + 1]
                            if d >= 0:
                                # diagonal corner tile: u-scaled copy then
                                # triangular mask on the pool engine
                                nc.scalar.activation(
                                    out=scores[:, cc, d:d + P],
                                    in_=ps[:, 0:P], func=AF.Copy, scale=ucol)
                                nc.gpsimd.tensor_mul(
                                    out=scores[:, cc, d:d + P],
                                    in0=scores[:, cc, d:d + P],
                                    in1=cmask[:, 384:512])
                                if d + P < 512:
                                    nc.vector.tensor_scalar_mul(
                                        out=scores[:, cc, d + P:512],
                                        in0=ps[:, P:nw], scalar1=ucol)
                            else:
                                if cc % 2 == 0:
                                    nc.scalar.activation(out=scores[:, cc, :],
                                                         in_=ps, func=AF.Copy,
                                                         scale=ucol)
                                else:
                                    nc.vector.tensor_scalar_mul(
                                        out=scores[:, cc, :], in0=ps,
                                        scalar1=ucol)
                        # scores @ v_aug per 128-row tile; normalizer + group
                        # norm batched over the 4 row-tiles of this t-chunk
                        hts = []
                        mv = smalls.tile([P, 4, 2], f32, name="mv", tag="mv", bufs=4)
                        bns = smalls.tile([P, 4, 6], f32, name="bns", tag="bns",
                                          bufs=4)
                        st_g = smalls.tile([P, 7, 4], f32, name="st_g", tag="st_g",
                                           bufs=4)
                        for ti in range(4):
                            t = tc_ * 4 + ti
                            pso = ps_sv.tile([P, DH + 1], f32, tag="sv", name="ps_sv")
                            for cc in range(t + 1):
                                nc.tensor.matmul(
                                    pso,
                                    scores[:, cc, ti * P:(ti + 1) * P],
                                    v_sb[:, cc, hd, :],
                                    start=(cc == 0), stop=(cc == t))
                            rcol = useq[:, t, 4 + hd:5 + hd]
                            # O~ = r * O_raw (bounded);  O~[:,DH] = r*S_raw
                            ht = smalls.tile([P, DH + 1], bf16, name="ht", tag="ht",
                                             bufs=cfg["ht_bufs"])
                            if ti % 2 == 0:
                                nc.scalar.activation(out=ht, in_=pso, func=AF.Copy,
                                                     scale=rcol)
                            else:
                                nc.vector.tensor_scalar_mul(out=ht, in0=pso,
                                                            scalar1=rcol)
                            hts.append(ht)
                            nc.gpsimd.tensor_copy(out=st_g[:, 0, ti:ti + 1],
                                                  in_=ht[:, DH:DH + 1])
                            nc.vector.bn_stats(out=bns[:, ti, :], in_=ht[:, 0:DH])
                            nc.vector.bn_aggr(out=mv[:, ti, :], in_=bns[:, ti, :])
                        em4 = useq[:, tc_ * 4:tc_ * 4 + 4, 8 + hd]
                        var4 = mv[:, :, 1]
                        mu4 = mv[:, :, 0]
                        # n = max(|r*S_raw|, em);  phi = rsqrt(var + eps*n^2)
                        nc.scalar.activation(out=st_g[:, 1, :], in_=st_g[:, 0, :],
                                             func=AF.Abs)
                        nc.vector.tensor_max(out=st_g[:, 2, :], in0=st_g[:, 1, :],
                                             in1=em4)
                        nc.gpsimd.tensor_mul(out=st_g[:, 3, :], in0=st_g[:, 2, :],
                                             in1=st_g[:, 2, :])
                        nc.vector.scalar_tensor_tensor(
                            out=st_g[:, 3, :], in0=st_g[:, 3, :], scalar=EPS,
                            in1=var4, op0=OP.mult, op1=OP.add)
                        nc.scalar.activation(out=st_g[:, 4, :], in_=st_g[:, 3, :],
                                             func=AF.Sqrt)
                        nc.vector.reciprocal(out=st_g[:, 5, :],
                                             in_=st_g[:, 4, :])  # phi
                        nc.vector.scalar_tensor_tensor(
                            out=st_g[:, 6, :], in0=mu4, scalar=-1.0,
                            in1=st_g[:, 5, :], op0=OP.mult, op1=OP.mult)
                        for ti in range(4):
                            t = tc_ * 4 + ti
                            if ti % 2 == 0:
                                nc.vector.tensor_scalar(
                                    out=hgn[:, t, hd * DH:(hd + 1) * DH],
                                    in0=hts[ti][:, 0:DH],
                                    scalar1=mv[:, ti, 0:1],
                                    scalar2=st_g[:, 5, ti:ti + 1],
                                    op0=OP.subtract, op1=OP.mult)
                            else:
                                nc.scalar.activation(
                                    out=hgn[:, t, hd * DH:(hd + 1) * DH],
                                    in_=hts[ti][:, 0:DH], func=AF.Identity,
                                    scale=st_g[:, 5, ti:ti + 1],
                                    bias=st_g[:, 6, ti:ti + 1])

                # ---------- transpose h_gn to feature-major + residual mix ----------
                hfin = act.tile([P, IC, S], f8, tag="qf", name="hfin")
                for h in range(HV):
                    for c in range(IC):
                        pst = ps_big.tile([P, 512], bf16, tag="big", name="ps_hT")
                        for k in range(4):
                            t = h * 4 + k
                            nc.tensor.transpose(
                                out=pst[:, k * P:(k + 1) * P],
                                in_=hgn[:, t, c * P:(c + 1) * P],
                                identity=id_bf)
                        post = smalls.tile([P, 512], bf16, name="post", tag="post",
                                           bufs=2)
                        # post = pst*gng*S_HF + csk  (csk already *S_HF)
                        nc.vector.scalar_tensor_tensor(
                            out=post, in0=pst, scalar=gng[:, l, c:c + 1],
                            in1=cact[:, c, h * 512:(h + 1) * 512],
                            op0=OP.mult, op1=OP.add)
                        nc.gpsimd.tensor_mul(
                            out=hfin[:, c, h * 512:(h + 1) * 512], in0=post,
                            in1=sz[:, c, h * 512:(h + 1) * 512])

                # ---------- down projection (fp8 DR) + bias matmul + residual ----------
                dnw = []
                for m in range(EC):
                    wt = wstream.tile([P, IC, P], f8, tag="wdown", name="wdown_t",
                                      bufs=4)
                    nc.sync.dma_start(out=wt, in_=wdown_d[l, m])
                    dnw.append(wt)
                for h in range(HV):
                    for m in range(EC):
                        wt = dnw[m]
                        ps = ps_big.tile([P, 512], f32, tag="big", name="ps_dn")
                        for c2 in range(IC // 2):
                            nc.tensor.matmul(ps, wt[:, 2 * c2:2 * c2 + 2, :],
                                             hfin[:, 2 * c2:2 * c2 + 2,
                                                  h * 512:(h + 1) * 512],
                                             perf_mode=DR,
                                             start=(c2 == 0), stop=False)
                        nc.tensor.matmul(ps, bdrow[:, l, m, :], onerow,
                                         start=False, stop=True)
                        nc.vector.scalar_tensor_tensor(
                            out=r_feat[:, m, h * 512:(h + 1) * 512], in0=ps,
                            scalar=1.0 / (S_HF * S_WD),
                            in1=r_feat[:, m, h * 512:(h + 1) * 512],
                            op0=OP.mult, op1=OP.add)

            # ================= final projection =================
            r_bf = act.tile([P, EC, S], bf16, tag="xnw", name="r_bf")
            yout = act.tile([P, ST, F], f32, tag="stage", name="yout")
            y_r = y_d.ap().rearrange("(t p) f -> p t f", p=P)
            for st in range(ST):
                nc.vector.tensor_copy(out=r_bf[:, :, st * P:(st + 1) * P],
                                      in_=r_feat[:, :, st * P:(st + 1) * P])
                ps = ps_big.tile([P, F], f32, tag="big", name="ps_fin")
                for c in range(EC):
                    nc.tensor.matmul(ps, r_bf[:, c, st * P:(st + 1) * P],
                                     wfin[:, c, :],
                                     start=(c == 0), stop=(c == EC - 1))
                nc.vector.tensor_add(out=yout[:, st, :], in0=ps, in1=bfin)
                if st % 2 == 1:
                    nc.sync.dma_start(out=y_r[:, st - 1:st + 1, :],
                                      in_=yout[:, st - 1:st + 1, :])

        for pool in (ps_sm, ps_sv, ps_big, smalls, wstream, act, rpool, const):
            pool.release()

    nc.compile()
    return nc


# ---------------------------------------------------------------------------
# entry point
# ---------------------------------------------------------------------------

_lock = threading.Lock()
_nc = None


def _get_nc():
    global _nc
    with _lock:
        if _nc is None:
            _nc = build_nc()
    return _nc


def _in_maps(inputs):
    w = _prep_weights(inputs)
    x = np.asarray(inputs["x"], np.float32)
    in_maps = []
    for b in range(NCORES):
        m = {"x": np.ascontiguousarray(x[b])}
        m.update(w)
        in_maps.append(m)
    return in_maps


def kernel(**inputs):
    nc = _get_nc()
    res = run_bass_kernel_spmd(nc, _in_maps(inputs),
                               core_ids=list(range(NCORES)))
    out = np.stack([res.results[b]["y"] for b in range(NCORES)], axis=0)
    return out.astype(np.float32)


def hw_time_ns(inputs, reps=20):
    """NTFF-profiled exec time (what the harness measures)."""
    nc = _get_nc()
    res = run_bass_kernel_spmd(nc, _in_maps(inputs),
                               core_ids=list(range(NCORES)), trace=True)
    return res.exec_time_ns, {"profile_json": res.profile_json}
